# revision 25
# baseline (speedup 1.0000x reference)
"""GATv2 layer on 8 Trainium2 NeuronCores (Bass/Tile).

Reference math (per batch b):
    hp = h @ lin_w.T + lin_b
    u  = hp @ W1.T ; v = hp @ W2.T          (W1, W2 = halves of W_w)
    e[i,j]   = sum_f a_f * LeakyReLU(u[i,f] + v[j,f])
    att      = softmax_j(where(adj, e, -inf))
    out      = elu(att @ hp)

Kernel decomposition (same algebra as the fp16 predecessor):
  a_f*LReLU(s) = alpha*a_f*s + (1-alpha)*sign(a_f)*relu(|a_f|*s); with
  u'' = |a|*u, v'' = |a|*v the alpha*su_i row term cancels in softmax and
  exp(alpha*sv_j) folds into the adjacency mask host-side.  The remaining
  work per (i, j) is the 64-term signed-relu contraction
      c[i,j] = sum_f sign(a_f) * relu(u''[i,f] + v''[j,f]).

  Mixed-precision f-split: features are ranked host-side by the second
  moment of u''+v''; the top 32 ("hot") contract in fp16, the bottom 32
  ("cold") in fp8e4m3 via DoubleRow matmuls.  Per quad of destinations a
  [128, 1024] hot tile (4 dests x 32 f stacked on partitions) feeds one
  fp16 matmul per 512-wide j-half; per octet a [128, 2048] cold tile (two
  quad k-tiles) feeds one DoubleRow matmul ([128, 2, 512] moving operand,
  256-deep contraction).  Row-shifted +-sign weight variants place each
  group's rows inside the [64, 512] PSUM half (matmul output base
  partitions are restricted to {0, 64}), so 16 hot + 8 DR matmuls
  accumulate one e half.  Relative error vs the fp32 reference: ~4e-3
  (fp8 tail features), inside the 2e-2 gate.

  Tile production: relu(vstack + ubias_col) via tensor_scalar /
  activation, split across DVE (hot fp16, 4x mode), ACT and GPSIMD (cold
  fp8) so production overlaps the PE stream.  exp(0.8*e) via ACT, masked
  by w_j*adj^T during the PSUM->SBUF copy after a PE transpose, then the
  PV matmul (attT @ [hp, 1]) yields numerator and denominator in one
  pass; divide + ELU epilogue.

Sharding: core c owns batch c//2, destination rows (c%2)*512 ... +512.
"""

import sys

import numpy as np

if "/opt/trn_rl_repo" not in sys.path:
    sys.path.insert(0, "/opt/trn_rl_repo")

ALPHA = 0.2
B, N, F = 4, 1024, 64
N_CORES = 8
ROWS_PER_CORE = B * N // N_CORES          # 512
BLK = 128
N_BLOCKS = ROWS_PER_CORE // BLK           # 4
N_JB = N // BLK                           # 8
HOT = 32                                  # fp16 features
COLD = F - HOT                            # fp8 features
QUADS_PER_HALF = 16                       # 64 dests / 4
OCTETS_PER_HALF = 8
N_QUADS = ROWS_PER_CORE // 4              # 128

_COMPILED = {}


def _cold_engines():
    """Engines for the 128 cold production instrs: 60 ACT, 48 GP, 20 DVE,
    interleaved by largest remainder so each engine's share arrives evenly."""
    quotas = {"scalar": 58.0, "gpsimd": 46.0, "vector": 24.0}
    acc = dict.fromkeys(quotas, 0.0)
    out = []
    for _ in range(128):
        for k in quotas:
            acc[k] += quotas[k] / 128.0
        pick = max(acc, key=lambda k: acc[k])
        acc[pick] -= 1.0
        out.append(pick)
    # octet 0's pair must run in parallel on the two earliest-ready engines
    # (ACT + DVE); GPSIMD starts later
    i = out.index("vector")
    out[1], out[i] = out[i], out[1]
    return out


_COLD_ENGINES = _cold_engines()


def _build_module():
    import concourse.tile as tile
    from concourse import bacc, mybir
    from contextlib import ExitStack

    f32 = mybir.dt.float32
    f16 = mybir.dt.float16
    f8 = mybir.dt.float8e4
    nc = bacc.Bacc("TRN2", target_bir_lowering=False, debug=False,
                   enable_asserts=True, num_devices=N_CORES)

    # merged vstacks (hot cols 0:1024, cold 1024:2048) — one HWDGE slot
    vs_ap = nc.dram_tensor("vs", (BLK, 2 * N), f16, kind="ExternalInput").ap()
    # first 16 quads' bias columns (hot cols 0-15, cold 16-31) in one small
    # early transfer so the first productions aren't gated
    ub0_ap = nc.dram_tensor("ub0", (BLK, 32), f32, kind="ExternalInput").ap()
    # merged late biases: hot quads 16-127 at cols 0:112, cold at 112:224
    ubhc_ap = nc.dram_tensor("ubhc", (BLK, 2 * (N_QUADS - 16)), f32,
                             kind="ExternalInput").ap()
    # hot sign variants: 16 x [128, 64] f16; variant t has s_hot at rows
    # 32*d .. 32*d+31 of column 4t+d
    sgnh_ap = nc.dram_tensor("sgnh", (BLK, QUADS_PER_HALF * 64), f16,
                             kind="ExternalInput").ap()
    # DR sign variants: 8 x [128, 2, 64] f8
    sgnc_ap = nc.dram_tensor("sgnc", (BLK, OCTETS_PER_HALF * 128), f8,
                             kind="ExternalInput").ap()
    ident_ap = nc.dram_tensor("ident", (BLK, BLK), f16, kind="ExternalInput").ap()
    # adjwt / hpx host-permuted so each lands in one [128, *] SBUF tile:
    # adjwt[p, jb*512 + i] = w_j * adj[i, j],  j = jb*128 + p
    # hpx[p, jb*65 + n]    = [hp | 1][j, n],   j = jb*128 + p
    adjwt_ap = nc.dram_tensor("adjwt", (BLK, N_JB * ROWS_PER_CORE), f16,
                              kind="ExternalInput").ap()
    hpx_ap = nc.dram_tensor("hpx", (BLK, N_JB * (F + 1)), f16,
                            kind="ExternalInput").ap()
    out_ap = nc.dram_tensor("out", (ROWS_PER_CORE, F), f32, kind="ExternalOutput").ap()

    Relu = mybir.ActivationFunctionType.Relu
    Exp = mybir.ActivationFunctionType.Exp
    add = mybir.AluOpType.add
    amax = mybir.AluOpType.max
    amin = mybir.AluOpType.min
    mult = mybir.AluOpType.mult

    with tile.TileContext(nc) as tc, ExitStack() as ctx:
        consts = ctx.enter_context(tc.tile_pool(name="consts", bufs=1))
        hpool = ctx.enter_context(tc.tile_pool(name="hpool", bufs=6))
        cpool = ctx.enter_context(tc.tile_pool(name="cpool", bufs=4))
        epool = ctx.enter_context(tc.tile_pool(name="epool", bufs=2))
        apool = ctx.enter_context(tc.tile_pool(name="apool", bufs=3))
        spool = ctx.enter_context(tc.tile_pool(name="spool", bufs=4))
        ps_e = ctx.enter_context(tc.tile_pool(name="ps_e", bufs=2, space="PSUM"))
        ps_t = ctx.enter_context(tc.tile_pool(name="ps_t", bufs=3, space="PSUM"))
        ps_h = ctx.enter_context(tc.tile_pool(name="ps_h", bufs=1, space="PSUM"))

        # Startup transfers. A DMA instruction holds its queue's sequencer
        # until the transfer's semaphore fires (~2.3us each), so each queue
        # carries exactly one early transfer: merged vstacks on SP, hot sign
        # weights on ACT, and the small/late ones on the SWDGE (gpsimd) path
        # which only costs Pool-engine descriptor generation.
        ub0 = consts.tile([BLK, 32], f32, tag="ub0")
        nc.gpsimd.dma_start(ub0[:], ub0_ap[:])
        vs = consts.tile([BLK, 2 * N], f16, tag="vs")
        nc.sync.dma_start(vs[:], vs_ap[:])
        vsh = vs[:, 0:N]
        vsc = vs[:, N:2 * N]
        sgnh = consts.tile([BLK, QUADS_PER_HALF * 64], f16, tag="sgnh")
        nc.scalar.dma_start(sgnh[:], sgnh_ap[:])
        sgnc = consts.tile([BLK, OCTETS_PER_HALF * 128], f8, tag="sgnc")
        nc.gpsimd.dma_start(sgnc[:], sgnc_ap[:])
        ident_t = consts.tile([BLK, BLK], f16, tag="ident")
        nc.gpsimd.dma_start(ident_t[:], ident_ap[:])
        ubhc = consts.tile([BLK, 2 * (N_QUADS - 16)], f32, tag="ubhc")
        adjwt = []
        hpx = []
        ident = []

        def load_aux():
            # issued after block 0's relu/matmul stream is underway so these
            # bulk transfers queue on the shared DMA belt behind the startup-
            # critical ones (SP + SWDGE; the ACT queue stays free)
            nc.sync.dma_start(ubhc[:], ubhc_ap[:])
            adjwt_t = consts.tile([BLK, N_JB * ROWS_PER_CORE], f16, tag="adjwt")
            nc.sync.dma_start(adjwt_t[:], adjwt_ap[:])
            hpx_t = consts.tile([BLK, N_JB * (F + 1)], f16, tag="hpx")
            nc.gpsimd.dma_start(hpx_t[:], hpx_ap[:])
            for jb in range(N_JB):
                adjwt.append(adjwt_t[:, jb * ROWS_PER_CORE:(jb + 1) * ROWS_PER_CORE])
                hpx.append(hpx_t[:, jb * (F + 1):(jb + 1) * (F + 1)])

        ident.append(ident_t)
        NQ16 = N_QUADS - 16

        def hbias(q):
            return (ub0[:, q:q + 1] if q < 16 else ubhc[:, q - 16:q - 15])

        def cbias(q):
            return (ub0[:, 16 + q:17 + q] if q < 16
                    else ubhc[:, NQ16 + q - 16:NQ16 + q - 15])

        # DoubleRow matmuls must write PSUM at partition base 0 (walrus emits
        # full-array col_grp for them), so the kernel processes 64-dest
        # blocks: every matmul output (DR, hot, transpose, PV) is base-0.
        cold_idx = 0
        for blk in range(2 * N_BLOCKS):              # 8 blocks of 64 dests
            e_ps = ps_e.tile([64, N], f32, tag="e")
            for o in range(OCTETS_PER_HALF):
                qa = blk * 16 + 2 * o                # global quad ids
                qb = qa + 1
                cold8 = cpool.tile([BLK, 2 * N], f8, tag="cold")
                for half, q in ((0, qa), (1, qb)):
                    eng = getattr(nc, _COLD_ENGINES[cold_idx])
                    cold_idx += 1
                    dst = cold8[:, half * N:(half + 1) * N]
                    if eng is nc.scalar:
                        nc.scalar.activation(dst, vsc[:], Relu,
                                             bias=cbias(q), scale=1.0)
                    else:
                        eng.tensor_scalar(dst, vsc[:], cbias(q), 0.0,
                                          op0=add, op1=amax)
                hotA = hpool.tile([BLK, N], f16, tag="hot")
                nc.vector.tensor_scalar(hotA[:], vsh[:], hbias(qa), 0.0,
                                        op0=add, op1=amax)
                hotB = hpool.tile([BLK, N], f16, tag="hot")
                nc.vector.tensor_scalar(hotB[:], vsh[:], hbias(qb), 0.0,
                                        op0=add, op1=amax)
                rhs3 = cold8[:].rearrange("p (t n) -> p t n", t=2)
                lw_dr = (sgnc[:, o * 128:(o + 1) * 128]
                         .rearrange("p (t m) -> p t m", t=2))
                lw_a = sgnh[:, (2 * o) * 64:(2 * o) * 64 + 64]
                lw_b = sgnh[:, (2 * o + 1) * 64:(2 * o + 1) * 64 + 64]
                for jh in range(2):
                    sl = slice(jh * 512, jh * 512 + 512)
                    out_sl = e_ps[:, sl]
                    nc.tensor.matmul(out_sl, lw_a, hotA[:, sl],
                                     start=(o == 0), stop=False)
                    nc.tensor.matmul(out_sl, lw_b, hotB[:, sl],
                                     start=False, stop=False)
                    nc.tensor.matmul(
                        out_sl, lw_dr, rhs3[:, :, sl],
                        start=False, stop=(o == OCTETS_PER_HALF - 1),
                        perf_mode=mybir.MatmulPerfMode.DoubleRow)
            if blk == 0:
                load_aux()
            # exp((1-alpha) * e), split in column halves so the first
            # transposes are not gated on the full pass
            exp_sb = epool.tile([64, N], f16, tag="exp")
            nc.scalar.activation(exp_sb[:, 0:512], e_ps[:, 0:512], Exp,
                                 scale=(1.0 - ALPHA))
            nc.scalar.activation(exp_sb[:, 512:1024], e_ps[:, 512:1024], Exp,
                                 scale=(1.0 - ALPHA))
            hnum = ps_h.tile([64, F + 1], f32, tag="hnum")
            for jb in range(N_JB):
                tp = ps_t.tile([BLK, 64], f16, tag="tp")
                nc.tensor.transpose(tp[:], exp_sb[:, jb * BLK:(jb + 1) * BLK],
                                    ident[0][0:64, 0:64])
                attT = apool.tile([BLK, 64], f16, tag="attT")
                nc.vector.tensor_mul(
                    attT[:], tp[:], adjwt[jb][:, blk * 64:(blk + 1) * 64])
                nc.tensor.matmul(hnum[:], attT[:], hpx[jb],
                                 start=(jb == 0), stop=(jb == N_JB - 1))
            # epilogue: h = num/den, out = elu(h) = relu(h) + exp(min(h,0)) - 1
            rec = spool.tile([64, 1], f32, tag="rec")
            nc.vector.reciprocal(rec[:], hnum[:, F:F + 1])
            m_t = spool.tile([64, F], f32, tag="m_t")
            nc.vector.tensor_scalar(m_t[:], hnum[:, 0:F], rec[:, 0:1], 0.0,
                                    op0=mult, op1=amin)
            g_t = spool.tile([64, F], f32, tag="g_t")
            nc.scalar.activation(g_t[:], m_t[:], Exp)
            r_t = spool.tile([64, F], f32, tag="r_t")
            nc.vector.tensor_scalar(r_t[:], hnum[:, 0:F], rec[:, 0:1], 0.0,
                                    op0=mult, op1=amax)
            o2 = spool.tile([64, F], f32, tag="o2")
            nc.vector.scalar_tensor_tensor(
                o2[:], r_t[:], -1.0, g_t[:], op0=add, op1=add)
            nc.sync.dma_start(out_ap[blk * 64:(blk + 1) * 64, :], o2[:])

    nc.finalize()
    return nc


def _host_precompute(h, adj, lin_w, lin_b, W_w, a):
    """Build per-core device input dicts (all small math in float64)."""
    import ml_dtypes
    f8 = ml_dtypes.float8_e4m3

    h64 = h.astype(np.float64)
    lin_w64 = lin_w.astype(np.float64)
    lin_b64 = lin_b.astype(np.float64)
    W1 = W_w[:, :F].astype(np.float64)
    W2 = W_w[:, F:].astype(np.float64)
    a64 = a[:, 0].astype(np.float64)

    M1 = W1 @ lin_w64
    c1 = W1 @ lin_b64
    M2 = W2 @ lin_w64
    c2 = W2 @ lin_b64
    aab = np.abs(a64)
    sgn_vec = np.sign(a64)
    ident = np.eye(BLK, dtype=np.float16)

    in_maps = []
    for c in range(N_CORES):
        b = c // 2
        r0 = (c % 2) * ROWS_PER_CORE
        hb = h64[b]                                        # [N, F]
        u = (hb @ M1.T + c1) * aab                         # u'' [N, F]
        v = (hb @ M2.T + c2) * aab                         # v'' [N, F]
        sv = v @ sgn_vec                                   # [N]
        w = np.exp(ALPHA * sv)                             # [N]
        hp = hb @ lin_w64.T + lin_b64                      # [N, F]

        # feature split by second moment of u + v
        mom = u.var(0) + v.var(0) + (u.mean(0) + v.mean(0)) ** 2
        order = np.argsort(-mom)
        hot_f, cold_f = order[:HOT], order[HOT:]
        s_hot, s_cold = sgn_vec[hot_f], sgn_vec[cold_f]

        v16 = v.astype(np.float16)
        vsh = np.tile(v16[:, hot_f].T, (4, 1)).astype(np.float16)   # [128, N]
        vsc = np.tile(v16[:, cold_f].T, (4, 1)).astype(np.float16)  # [128, N]

        # per-quad bias columns: quad q covers dests r0 + 4q + d, d = row//32
        uc = u[r0:r0 + ROWS_PER_CORE]                      # [512, F]
        ubh = np.empty((BLK, N_QUADS), dtype=np.float32)
        ubc = np.empty((BLK, N_QUADS), dtype=np.float32)
        for d in range(4):
            ubh[d * 32:(d + 1) * 32, :] = uc[d::4, :][:, hot_f].T
            ubc[d * 32:(d + 1) * 32, :] = uc[d::4, :][:, cold_f].T

        # hot sign variants: 16 x [128, 64]
        sgnh = np.zeros((BLK, QUADS_PER_HALF, 64), dtype=np.float16)
        for t in range(QUADS_PER_HALF):
            for d in range(4):
                sgnh[d * 32:(d + 1) * 32, t, 4 * t + d] = s_hot
        sgnh = sgnh.reshape(BLK, QUADS_PER_HALF * 64)

        # DR sign variants: 8 x [128, 2, 64]
        sgnc = np.zeros((BLK, OCTETS_PER_HALF, 2, 64), dtype=f8)
        for o in range(OCTETS_PER_HALF):
            for k in range(2):
                for d in range(4):
                    sgnc[d * 32:(d + 1) * 32, o, k, 8 * o + 4 * k + d] = \
                        s_cold.astype(f8)
        sgnc = sgnc.reshape(BLK, OCTETS_PER_HALF * 128)

        adjwt = (adj[b, r0:r0 + ROWS_PER_CORE, :].T.astype(np.float64)
                 * w[:, None]).astype(np.float16)          # [N, 512]
        adjwt = adjwt.reshape(N_JB, BLK, ROWS_PER_CORE).transpose(1, 0, 2)
        adjwt = adjwt.reshape(BLK, N_JB * ROWS_PER_CORE)
        hpx = np.concatenate(
            [hp, np.ones((N, 1))], axis=1).astype(np.float16)  # [N, 65]
        hpx = hpx.reshape(N_JB, BLK, F + 1).transpose(1, 0, 2)
        hpx = hpx.reshape(BLK, N_JB * (F + 1))

        in_maps.append({
            "vs": np.ascontiguousarray(
                np.concatenate([vsh, vsc], axis=1)),
            "ub0": np.ascontiguousarray(
                np.concatenate([ubh[:, :16], ubc[:, :16]], axis=1)),
            "ubhc": np.ascontiguousarray(
                np.concatenate([ubh[:, 16:], ubc[:, 16:]], axis=1)),
            "sgnh": sgnh,
            "sgnc": sgnc,
            "adjwt": np.ascontiguousarray(adjwt),
            "hpx": np.ascontiguousarray(hpx),
            "ident": ident,
        })
    return in_maps


def kernel(h, adj, lin_w, lin_b, W_w, a):
    from concourse.bass_utils import run_bass_kernel_spmd

    h, adj, lin_w, lin_b, W_w, a = (
        np.asarray(x) for x in (h, adj, lin_w, lin_b, W_w, a))

    if "nc" not in _COMPILED:
        _COMPILED["nc"] = _build_module()
    nc = _COMPILED["nc"]

    in_maps = _host_precompute(h, adj, lin_w, lin_b, W_w, a)
    res = run_bass_kernel_spmd(nc, in_maps, core_ids=list(range(N_CORES)))

    out = np.empty((B, N, F), dtype=np.float32)
    for c in range(N_CORES):
        b = c // 2
        r0 = (c % 2) * ROWS_PER_CORE
        out[b, r0:r0 + ROWS_PER_CORE, :] = res.results[c]["out"]
    return out


# revision 27
# speedup vs baseline: 1.0402x; 1.0402x over previous
"""GATv2 layer on 8 Trainium2 NeuronCores (Bass/Tile).

Reference math (per batch b):
    hp = h @ lin_w.T + lin_b
    u  = hp @ W1.T ; v = hp @ W2.T          (W1, W2 = halves of W_w)
    e[i,j]   = sum_f a_f * LeakyReLU(u[i,f] + v[j,f])
    att      = softmax_j(where(adj, e, -inf))
    out      = elu(att @ hp)

Kernel decomposition (same algebra as the fp16 predecessor):
  a_f*LReLU(s) = alpha*a_f*s + (1-alpha)*sign(a_f)*relu(|a_f|*s); with
  u'' = |a|*u, v'' = |a|*v the alpha*su_i row term cancels in softmax and
  exp(alpha*sv_j) folds into the adjacency mask host-side.  The remaining
  work per (i, j) is the 64-term signed-relu contraction
      c[i,j] = sum_f sign(a_f) * relu(u''[i,f] + v''[j,f]).

  Mixed-precision f-split: features are ranked host-side by the second
  moment of u''+v''; the top 32 ("hot") contract in fp16, the bottom 32
  ("cold") in fp8e4m3 via DoubleRow matmuls.  Per quad of destinations a
  [128, 1024] hot tile (4 dests x 32 f stacked on partitions) feeds one
  fp16 matmul per 512-wide j-half; per octet a [128, 2048] cold tile (two
  quad k-tiles) feeds one DoubleRow matmul ([128, 2, 512] moving operand,
  256-deep contraction).  Row-shifted +-sign weight variants place each
  group's rows inside the [64, 512] PSUM half (matmul output base
  partitions are restricted to {0, 64}), so 16 hot + 8 DR matmuls
  accumulate one e half.  Relative error vs the fp32 reference: ~4e-3
  (fp8 tail features), inside the 2e-2 gate.

  Tile production: relu(vstack + ubias_col) via tensor_scalar /
  activation, split across DVE (hot fp16, 4x mode), ACT and GPSIMD (cold
  fp8) so production overlaps the PE stream.  exp(0.8*e) via ACT, masked
  by w_j*adj^T during the PSUM->SBUF copy after a PE transpose, then the
  PV matmul (attT @ [hp, 1]) yields numerator and denominator in one
  pass; divide + ELU epilogue.

Sharding: core c owns batch c//2, destination rows (c%2)*512 ... +512.
"""

import sys

import numpy as np

if "/opt/trn_rl_repo" not in sys.path:
    sys.path.insert(0, "/opt/trn_rl_repo")

ALPHA = 0.2
B, N, F = 4, 1024, 64
N_CORES = 8
ROWS_PER_CORE = B * N // N_CORES          # 512
BLK = 128
N_BLOCKS = ROWS_PER_CORE // BLK           # 4
N_JB = N // BLK                           # 8
HOT = 32                                  # fp16 features
COLD = F - HOT                            # fp8 features
QUADS_PER_HALF = 16                       # 64 dests / 4
OCTETS_PER_HALF = 8
N_QUADS = ROWS_PER_CORE // 4              # 128

_COMPILED = {}


def _cold_engines():
    """Engines for the 128 cold production instrs: 60 ACT, 48 GP, 20 DVE,
    interleaved by largest remainder so each engine's share arrives evenly."""
    quotas = {"scalar": 58.0, "gpsimd": 46.0, "vector": 24.0}
    acc = dict.fromkeys(quotas, 0.0)
    out = []
    for _ in range(128):
        for k in quotas:
            acc[k] += quotas[k] / 128.0
        pick = max(acc, key=lambda k: acc[k])
        acc[pick] -= 1.0
        out.append(pick)
    # octet 0's pair must run in parallel on the two earliest-ready engines
    # (ACT + DVE); GPSIMD starts later
    i = out.index("vector")
    out[1], out[i] = out[i], out[1]
    return out


_COLD_ENGINES = _cold_engines()


def _build_module():
    import concourse.tile as tile
    from concourse import bacc, mybir
    from contextlib import ExitStack

    f32 = mybir.dt.float32
    f16 = mybir.dt.float16
    f8 = mybir.dt.float8e4
    nc = bacc.Bacc("TRN2", target_bir_lowering=False, debug=False,
                   enable_asserts=True, num_devices=N_CORES)

    # merged vstacks (hot cols 0:1024, cold 1024:2048) — one HWDGE slot
    vs_ap = nc.dram_tensor("vs", (BLK, 2 * N), f16, kind="ExternalInput").ap()
    # first 16 quads' bias columns (hot cols 0-15, cold 16-31) in one small
    # early transfer so the first productions aren't gated
    ub0_ap = nc.dram_tensor("ub0", (BLK, 32), f32, kind="ExternalInput").ap()
    # merged late biases: hot quads 16-127 at cols 0:112, cold at 112:224
    ubhc_ap = nc.dram_tensor("ubhc", (BLK, 2 * (N_QUADS - 16)), f32,
                             kind="ExternalInput").ap()
    # hot sign variants: 16 x [128, 64] f16; variant t has s_hot at rows
    # 32*d .. 32*d+31 of column 4t+d
    sgnh_ap = nc.dram_tensor("sgnh", (BLK, QUADS_PER_HALF * 64), f16,
                             kind="ExternalInput").ap()
    # DR sign variants: 8 x [128, 2, 64] f8
    sgnc_ap = nc.dram_tensor("sgnc", (BLK, OCTETS_PER_HALF * 128), f8,
                             kind="ExternalInput").ap()
    ident_ap = nc.dram_tensor("ident", (BLK, BLK), f16, kind="ExternalInput").ap()
    # adjwt / hpx host-permuted so each lands in one [128, *] SBUF tile:
    # adjwt[p, jb*512 + i] = w_j * adj[i, j],  j = jb*128 + p
    # hpx[p, jb*65 + n]    = [hp | 1][j, n],   j = jb*128 + p
    adjwt_ap = nc.dram_tensor("adjwt", (BLK, N_JB * ROWS_PER_CORE), f16,
                              kind="ExternalInput").ap()
    hpx_ap = nc.dram_tensor("hpx", (BLK, N_JB * (F + 1)), f16,
                            kind="ExternalInput").ap()
    out_ap = nc.dram_tensor("out", (ROWS_PER_CORE, F), f32, kind="ExternalOutput").ap()

    Relu = mybir.ActivationFunctionType.Relu
    Exp = mybir.ActivationFunctionType.Exp
    add = mybir.AluOpType.add
    amax = mybir.AluOpType.max
    amin = mybir.AluOpType.min
    mult = mybir.AluOpType.mult

    with tile.TileContext(nc) as tc, ExitStack() as ctx:
        consts = ctx.enter_context(tc.tile_pool(name="consts", bufs=1))
        hpool = ctx.enter_context(tc.tile_pool(name="hpool", bufs=6))
        cpool = ctx.enter_context(tc.tile_pool(name="cpool", bufs=4))
        epool = ctx.enter_context(tc.tile_pool(name="epool", bufs=2))
        apool = ctx.enter_context(tc.tile_pool(name="apool", bufs=3))
        spool = ctx.enter_context(tc.tile_pool(name="spool", bufs=4))
        ps_e = ctx.enter_context(tc.tile_pool(name="ps_e", bufs=2, space="PSUM"))
        ps_t = ctx.enter_context(tc.tile_pool(name="ps_t", bufs=3, space="PSUM"))
        ps_h = ctx.enter_context(tc.tile_pool(name="ps_h", bufs=1, space="PSUM"))

        # Startup transfers. A DMA instruction holds its queue's sequencer
        # until the transfer's semaphore fires (~2.3us each), so each queue
        # carries exactly one early transfer: merged vstacks on SP, hot sign
        # weights on ACT, and the small/late ones on the SWDGE (gpsimd) path
        # which only costs Pool-engine descriptor generation.
        ub0 = consts.tile([BLK, 32], f32, tag="ub0")
        nc.gpsimd.dma_start(ub0[:], ub0_ap[:])
        vs = consts.tile([BLK, 2 * N], f16, tag="vs")
        nc.sync.dma_start(vs[:], vs_ap[:])
        vsh = vs[:, 0:N]
        vsc = vs[:, N:2 * N]
        sgnh = consts.tile([BLK, QUADS_PER_HALF * 64], f16, tag="sgnh")
        nc.scalar.dma_start(sgnh[:], sgnh_ap[:])
        sgnc = consts.tile([BLK, OCTETS_PER_HALF * 128], f8, tag="sgnc")
        nc.gpsimd.dma_start(sgnc[:], sgnc_ap[:])
        ident_t = consts.tile([BLK, BLK], f16, tag="ident")
        nc.gpsimd.dma_start(ident_t[:], ident_ap[:])
        ubhc = consts.tile([BLK, 2 * (N_QUADS - 16)], f32, tag="ubhc")
        adjwt = []
        hpx = []
        ident = []

        def load_aux():
            # issued after block 0's relu/matmul stream is underway so these
            # bulk transfers queue on the shared DMA belt behind the startup-
            # critical ones (SP + SWDGE; the ACT queue stays free)
            nc.sync.dma_start(ubhc[:], ubhc_ap[:])
            adjwt_t = consts.tile([BLK, N_JB * ROWS_PER_CORE], f16, tag="adjwt")
            nc.sync.dma_start(adjwt_t[:], adjwt_ap[:])
            hpx_t = consts.tile([BLK, N_JB * (F + 1)], f16, tag="hpx")
            nc.gpsimd.dma_start(hpx_t[:], hpx_ap[:])
            for jb in range(N_JB):
                adjwt.append(adjwt_t[:, jb * ROWS_PER_CORE:(jb + 1) * ROWS_PER_CORE])
                hpx.append(hpx_t[:, jb * (F + 1):(jb + 1) * (F + 1)])

        ident.append(ident_t)
        NQ16 = N_QUADS - 16

        # PE p-state warmup: a stream of tiny matmuls from ~t=0.3us keeps the
        # tensor engine's ramp clock running so the first real matmuls
        # dispatch at full rate. Count tuned so the stream drains just as the
        # first production tiles land.
        wsrc = consts.tile([BLK, 16], f16, tag="wsrc")
        nc.vector.memset(wsrc[:], 1.0)
        wps = ps_h.tile([64, F + 1], f32, tag="hnum")
        for _ in range(220):
            nc.tensor.matmul(wps[0:4, 0:8], wsrc[:, 0:4], wsrc[:, 8:16],
                             start=True, stop=True)

        def hbias(q):
            return (ub0[:, q:q + 1] if q < 16 else ubhc[:, q - 16:q - 15])

        def cbias(q):
            return (ub0[:, 16 + q:17 + q] if q < 16
                    else ubhc[:, NQ16 + q - 16:NQ16 + q - 15])

        # DoubleRow matmuls must write PSUM at partition base 0 (walrus emits
        # full-array col_grp for them), so the kernel processes 64-dest
        # blocks: every matmul output (DR, hot, transpose, PV) is base-0.
        cold_idx = 0
        for blk in range(2 * N_BLOCKS):              # 8 blocks of 64 dests
            e_ps = ps_e.tile([64, N], f32, tag="e")
            for o in range(OCTETS_PER_HALF):
                qa = blk * 16 + 2 * o                # global quad ids
                qb = qa + 1
                cold8 = cpool.tile([BLK, 2 * N], f8, tag="cold")
                for half, q in ((0, qa), (1, qb)):
                    eng = getattr(nc, _COLD_ENGINES[cold_idx])
                    cold_idx += 1
                    dst = cold8[:, half * N:(half + 1) * N]
                    if eng is nc.scalar:
                        nc.scalar.activation(dst, vsc[:], Relu,
                                             bias=cbias(q), scale=1.0)
                    else:
                        eng.tensor_scalar(dst, vsc[:], cbias(q), 0.0,
                                          op0=add, op1=amax)
                hotA = hpool.tile([BLK, N], f16, tag="hot")
                nc.vector.tensor_scalar(hotA[:], vsh[:], hbias(qa), 0.0,
                                        op0=add, op1=amax)
                hotB = hpool.tile([BLK, N], f16, tag="hot")
                nc.vector.tensor_scalar(hotB[:], vsh[:], hbias(qb), 0.0,
                                        op0=add, op1=amax)
                rhs3 = cold8[:].rearrange("p (t n) -> p t n", t=2)
                lw_dr = (sgnc[:, o * 128:(o + 1) * 128]
                         .rearrange("p (t m) -> p t m", t=2))
                lw_a = sgnh[:, (2 * o) * 64:(2 * o) * 64 + 64]
                lw_b = sgnh[:, (2 * o + 1) * 64:(2 * o + 1) * 64 + 64]
                for jh in range(2):
                    sl = slice(jh * 512, jh * 512 + 512)
                    out_sl = e_ps[:, sl]
                    nc.tensor.matmul(out_sl, lw_a, hotA[:, sl],
                                     start=(o == 0), stop=False)
                    nc.tensor.matmul(out_sl, lw_b, hotB[:, sl],
                                     start=False, stop=False)
                    nc.tensor.matmul(
                        out_sl, lw_dr, rhs3[:, :, sl],
                        start=False, stop=(o == OCTETS_PER_HALF - 1),
                        perf_mode=mybir.MatmulPerfMode.DoubleRow)
            if blk == 0:
                load_aux()
            # exp((1-alpha) * e), split in column halves so the first
            # transposes are not gated on the full pass
            exp_sb = epool.tile([64, N], f16, tag="exp")
            nc.scalar.activation(exp_sb[:, 0:512], e_ps[:, 0:512], Exp,
                                 scale=(1.0 - ALPHA))
            nc.scalar.activation(exp_sb[:, 512:1024], e_ps[:, 512:1024], Exp,
                                 scale=(1.0 - ALPHA))
            hnum = ps_h.tile([64, F + 1], f32, tag="hnum")
            for jb in range(N_JB):
                tp = ps_t.tile([BLK, 64], f16, tag="tp")
                nc.tensor.transpose(tp[:], exp_sb[:, jb * BLK:(jb + 1) * BLK],
                                    ident[0][0:64, 0:64])
                attT = apool.tile([BLK, 64], f16, tag="attT")
                nc.vector.tensor_mul(
                    attT[:], tp[:], adjwt[jb][:, blk * 64:(blk + 1) * 64])
                nc.tensor.matmul(hnum[:], attT[:], hpx[jb],
                                 start=(jb == 0), stop=(jb == N_JB - 1))
            # epilogue: h = num/den, out = elu(h) = relu(h) + exp(min(h,0)) - 1
            rec = spool.tile([64, 1], f32, tag="rec")
            nc.vector.reciprocal(rec[:], hnum[:, F:F + 1])
            m_t = spool.tile([64, F], f32, tag="m_t")
            nc.vector.tensor_scalar(m_t[:], hnum[:, 0:F], rec[:, 0:1], 0.0,
                                    op0=mult, op1=amin)
            g_t = spool.tile([64, F], f32, tag="g_t")
            nc.scalar.activation(g_t[:], m_t[:], Exp)
            r_t = spool.tile([64, F], f32, tag="r_t")
            nc.vector.tensor_scalar(r_t[:], hnum[:, 0:F], rec[:, 0:1], 0.0,
                                    op0=mult, op1=amax)
            o2 = spool.tile([64, F], f32, tag="o2")
            nc.vector.scalar_tensor_tensor(
                o2[:], r_t[:], -1.0, g_t[:], op0=add, op1=add)
            nc.sync.dma_start(out_ap[blk * 64:(blk + 1) * 64, :], o2[:])

    nc.finalize()
    return nc


def _host_precompute(h, adj, lin_w, lin_b, W_w, a):
    """Build per-core device input dicts (all small math in float64)."""
    import ml_dtypes
    f8 = ml_dtypes.float8_e4m3

    h64 = h.astype(np.float64)
    lin_w64 = lin_w.astype(np.float64)
    lin_b64 = lin_b.astype(np.float64)
    W1 = W_w[:, :F].astype(np.float64)
    W2 = W_w[:, F:].astype(np.float64)
    a64 = a[:, 0].astype(np.float64)

    M1 = W1 @ lin_w64
    c1 = W1 @ lin_b64
    M2 = W2 @ lin_w64
    c2 = W2 @ lin_b64
    aab = np.abs(a64)
    sgn_vec = np.sign(a64)
    ident = np.eye(BLK, dtype=np.float16)

    in_maps = []
    for c in range(N_CORES):
        b = c // 2
        r0 = (c % 2) * ROWS_PER_CORE
        hb = h64[b]                                        # [N, F]
        u = (hb @ M1.T + c1) * aab                         # u'' [N, F]
        v = (hb @ M2.T + c2) * aab                         # v'' [N, F]
        sv = v @ sgn_vec                                   # [N]
        w = np.exp(ALPHA * sv)                             # [N]
        hp = hb @ lin_w64.T + lin_b64                      # [N, F]

        # feature split by second moment of u + v
        mom = u.var(0) + v.var(0) + (u.mean(0) + v.mean(0)) ** 2
        order = np.argsort(-mom)
        hot_f, cold_f = order[:HOT], order[HOT:]
        s_hot, s_cold = sgn_vec[hot_f], sgn_vec[cold_f]

        v16 = v.astype(np.float16)
        vsh = np.tile(v16[:, hot_f].T, (4, 1)).astype(np.float16)   # [128, N]
        vsc = np.tile(v16[:, cold_f].T, (4, 1)).astype(np.float16)  # [128, N]

        # per-quad bias columns: quad q covers dests r0 + 4q + d, d = row//32
        uc = u[r0:r0 + ROWS_PER_CORE]                      # [512, F]
        ubh = np.empty((BLK, N_QUADS), dtype=np.float32)
        ubc = np.empty((BLK, N_QUADS), dtype=np.float32)
        for d in range(4):
            ubh[d * 32:(d + 1) * 32, :] = uc[d::4, :][:, hot_f].T
            ubc[d * 32:(d + 1) * 32, :] = uc[d::4, :][:, cold_f].T

        # hot sign variants: 16 x [128, 64]
        sgnh = np.zeros((BLK, QUADS_PER_HALF, 64), dtype=np.float16)
        for t in range(QUADS_PER_HALF):
            for d in range(4):
                sgnh[d * 32:(d + 1) * 32, t, 4 * t + d] = s_hot
        sgnh = sgnh.reshape(BLK, QUADS_PER_HALF * 64)

        # DR sign variants: 8 x [128, 2, 64]
        sgnc = np.zeros((BLK, OCTETS_PER_HALF, 2, 64), dtype=f8)
        for o in range(OCTETS_PER_HALF):
            for k in range(2):
                for d in range(4):
                    sgnc[d * 32:(d + 1) * 32, o, k, 8 * o + 4 * k + d] = \
                        s_cold.astype(f8)
        sgnc = sgnc.reshape(BLK, OCTETS_PER_HALF * 128)

        adjwt = (adj[b, r0:r0 + ROWS_PER_CORE, :].T.astype(np.float64)
                 * w[:, None]).astype(np.float16)          # [N, 512]
        adjwt = adjwt.reshape(N_JB, BLK, ROWS_PER_CORE).transpose(1, 0, 2)
        adjwt = adjwt.reshape(BLK, N_JB * ROWS_PER_CORE)
        hpx = np.concatenate(
            [hp, np.ones((N, 1))], axis=1).astype(np.float16)  # [N, 65]
        hpx = hpx.reshape(N_JB, BLK, F + 1).transpose(1, 0, 2)
        hpx = hpx.reshape(BLK, N_JB * (F + 1))

        in_maps.append({
            "vs": np.ascontiguousarray(
                np.concatenate([vsh, vsc], axis=1)),
            "ub0": np.ascontiguousarray(
                np.concatenate([ubh[:, :16], ubc[:, :16]], axis=1)),
            "ubhc": np.ascontiguousarray(
                np.concatenate([ubh[:, 16:], ubc[:, 16:]], axis=1)),
            "sgnh": sgnh,
            "sgnc": sgnc,
            "adjwt": np.ascontiguousarray(adjwt),
            "hpx": np.ascontiguousarray(hpx),
            "ident": ident,
        })
    return in_maps


def kernel(h, adj, lin_w, lin_b, W_w, a):
    from concourse.bass_utils import run_bass_kernel_spmd

    h, adj, lin_w, lin_b, W_w, a = (
        np.asarray(x) for x in (h, adj, lin_w, lin_b, W_w, a))

    if "nc" not in _COMPILED:
        _COMPILED["nc"] = _build_module()
    nc = _COMPILED["nc"]

    in_maps = _host_precompute(h, adj, lin_w, lin_b, W_w, a)
    res = run_bass_kernel_spmd(nc, in_maps, core_ids=list(range(N_CORES)))

    out = np.empty((B, N, F), dtype=np.float32)
    for c in range(N_CORES):
        b = c // 2
        r0 = (c % 2) * ROWS_PER_CORE
        out[b, r0:r0 + ROWS_PER_CORE, :] = res.results[c]["out"]
    return out
